# revision 28
# baseline (speedup 1.0000x reference)
"""AlignedTargetsLoss (CTC forced-alignment Viterbi loss) on 8 TRN2 NeuronCores.

Key algebraic reduction: the masked-mean NLL of the Viterbi-aligned path equals
-(best path score)/count, and the best path score decomposes as
    score_b = PB_b + D[L_b-1][T_b-1]
where PB_b = sum_{t<T_b} (logits[t,0] - lse[t])  (blank log-prob prefix) and
D/E is a row DP over labels u (intervals formulation of the CTC state graph):
    E[u][t] = g_u[t] + max(E[u][t-1], P_u[t]),   g_u[t] = logits[t,y_u]-logits[t,0]
    P_u[t]  = max(D[u-1][t-1] + repneg_u, D[u-1][t-2]),  repneg = -inf if y_u==y_{u-1}
    D[u][t] = max(D[u][t-1], E[u][t])
E-scan and D-scan each map to one hardware tensor_tensor_scan instruction.
No backtrace needed: the loss only needs the path score.

g is produced on-device via PE transpose + one-hot matmul (the one-hot also
bakes in the -logits[:,0] subtraction), staged through DRAM, and streamed back
row-by-row for the DP. Small index-derived tables (one-hots, masks) are
precomputed on host. Sharding: pure data parallelism, 8 examples per core; the
host sums the per-core partial scores and divides by the total frame count.
"""

import os
import sys

sys.path.insert(0, "/opt/trn_rl_repo")

import numpy as np

B, T, V, U = 64, 2048, 256, 256
NCORES = 8
BSH = B // NCORES  # 8 examples per core
NTB = 16  # t-blocks of 128
TBS = T // NTB  # 128
NDIAG = U + 2 * (NTB - 1)  # 286 wavefront diagonals
NEG = -1.0e30

_cached = {}


def _build():
    import concourse.bass as bass
    import concourse.bacc as bacc
    import concourse.mybir as mybir
    from concourse.tile import TileContext

    f32 = mybir.dt.float32
    bf16 = mybir.dt.bfloat16
    AF = mybir.ActivationFunctionType
    OP = mybir.AluOpType

    nc = bacc.Bacc()

    logits_e = nc.declare_dram_parameter("logits", [BSH, T, V], f32, isOutput=False)
    oh_e = nc.declare_dram_parameter("oh", [BSH, 128, 2 * U], bf16, isOutput=False)
    ident_e = nc.declare_dram_parameter("ident", [128, 128], bf16, isOutput=False)
    rn_e = nc.declare_dram_parameter("rn", [128, NDIAG], f32, isOutput=False)
    ln_e = nc.declare_dram_parameter("ln", [128, NDIAG], f32, isOutput=False)
    zz_e = nc.declare_dram_parameter("zz", [128, NDIAG], f32, isOutput=False)
    bnc_e = nc.declare_dram_parameter("bnc", [128, 1], f32, isOutput=False)
    emb_e = nc.declare_dram_parameter("emb", [BSH, 128, NTB], f32, isOutput=False)
    pbm_e = nc.declare_dram_parameter("pbm", [BSH, 128, NTB], f32, isOutput=False)
    ones_e = nc.declare_dram_parameter("ones", [128, 1], f32, isOutput=False)
    out_e = nc.declare_dram_parameter("out", [1], f32, isOutput=True)

    with TileContext(nc) as tc:
        import contextlib

        ctx = contextlib.ExitStack()
        with ctx:
            dramp = ctx.enter_context(tc.tile_pool(name="dram", bufs=1, space="DRAM"))
            cpool = ctx.enter_context(tc.tile_pool(name="const", bufs=1))
            lpool = ctx.enter_context(tc.tile_pool(name="logit", bufs=3))
            tpool = ctx.enter_context(tc.tile_pool(name="tmp", bufs=3))
            ppool = ctx.enter_context(tc.tile_pool(name="psum", bufs=2, space="PSUM"))
            fpool = ctx.enter_context(tc.tile_pool(name="fin", bufs=1, space="PSUM"))
            gpool = ctx.enter_context(tc.tile_pool(name="grow", bufs=4))
            dpool = ctx.enter_context(tc.tile_pool(name="dp", bufs=1))

            g_dram = dramp.tile([BSH, NDIAG, T], f32)
            f_dram = dramp.tile([128], f32)

            # ---- constant tables from host ----
            rnTab = cpool.tile([128, NDIAG], f32)
            nc.sync.dma_start(out=rnTab[:], in_=rn_e[:])
            lnTab = cpool.tile([128, NDIAG], f32)
            nc.sync.dma_start(out=lnTab[:], in_=ln_e[:])
            zzTab = cpool.tile([128, NDIAG], f32)
            nc.sync.dma_start(out=zzTab[:], in_=zz_e[:])
            bnc = cpool.tile([128, 1], f32)
            nc.sync.dma_start(out=bnc[:], in_=bnc_e[:])
            ident = cpool.tile([128, 128], bf16)
            nc.sync.dma_start(out=ident[:], in_=ident_e[:])
            ones128 = cpool.tile([128, 1], f32)
            nc.sync.dma_start(out=ones128[:], in_=ones_e[:])
            ohs = []
            for ex in range(BSH):
                oh = cpool.tile([128, 2 * U], bf16, tag=f"oh{ex}", name=f"oh{ex}")
                nc.sync.dma_start(out=oh[:], in_=oh_e[ex])
                ohs.append(oh)
            embs = []
            for ex in range(BSH):
                eb = cpool.tile([128, NTB], f32, tag=f"eb{ex}", name=f"eb{ex}")
                nc.sync.dma_start(out=eb[:], in_=emb_e[ex])
                embs.append(eb)

            # zero-fill ONLY the never-written g_dram band: rows [0,32) and
            # [256,286) (everything else is covered by phase A stores).
            zrow = cpool.tile([128, T], f32)
            nc.vector.memset(zrow[:], 0.0)
            for lo, hi in ((0, 32), (256, NDIAG)):
                for ex in range(BSH):
                    nc.gpsimd.dma_start(
                        out=g_dram[ex, lo:hi, :], in_=zrow[0 : hi - lo, :]
                    )

            sgrids = []
            for ex in range(BSH):
                sg = cpool.tile([128, NTB], f32, tag=f"sg{ex}", name=f"sg{ex}")
                sgrids.append(sg)

            # ---- DP state (phase B wavefront over (row u, t-block) diagonals)
            # partition p = ex*16 + tb; tile (u, tb) processed at d = u + 2*tb.
            # ring[i]: col 0 = E-carry-in, cols 1:3 = D-halo guards (D[-2], D[-1]),
            # cols 3:131 = this row-tile's D values.
            rings = []
            for i in range(3):
                rg = dpool.tile([128, 131], f32, tag=f"ring{i}", name=f"ring{i}")
                rings.append(rg)
                nc.vector.memset(rg[:, 0:3], NEG)
                nc.vector.memset(rg[:, 3:131], 0.0)
            ets = []
            for i in range(2):
                et = dpool.tile([128, TBS], f32, tag=f"et{i}", name=f"et{i}")
                ets.append(et)
            pt = dpool.tile([128, TBS], f32)
            acc = dpool.tile([128, 1], f32)
            nc.vector.memset(acc[:], NEG)
            shuf_mask = [i if i % 16 == 0 else i - 1 for i in range(32)]

            def phase_b_diag(d):
                g_t = gpool.tile([128, TBS], f32, tag="g_t")
                nc.sync.dma_start(
                    out=g_t[:],
                    in_=g_dram[:, d, :].rearrange("e (k t) -> e k t", k=NTB),
                )
                rp = rings[(d + 2) % 3]   # prev-row buffer (written at d-1)
                rc = rings[d % 3]         # current buffer (written now)
                et = ets[d % 2]
                # P = max(max(Dprev<<1 + rn, Dprev<<2), z)
                nc.vector.scalar_tensor_tensor(
                    pt[:], rp[:, 2:130], rnTab[:, d : d + 1], rp[:, 1:129],
                    OP.add, OP.max,
                )
                if d <= 2 * (NTB - 1) and d % 2 == 0:
                    nc.vector.tensor_scalar_max(
                        pt[:], pt[:], zzTab[:, d : d + 1]
                    )
                # E scan; carry-in at ring col 0 (shipped from left tile at d-2)
                nc.vector.tensor_tensor_scan(
                    et[:], pt[:], g_t[:], rc[:, 0:1], OP.max, OP.add
                )
                # D scan; carry-in = D[-1] guard (col 2, shipped at d-2)
                nc.vector.tensor_tensor_scan(
                    rc[:, 3:131], et[:], et[:], rc[:, 2:3], OP.max, OP.max
                )
                if d >= 127:
                    # emb poisons g beyond each example's last valid frame, so
                    # D[:,127] == D at the final frame; ln selects u == L-1.
                    nc.vector.scalar_tensor_tensor(
                        acc[:], rc[:, 130:131], lnTab[:, d : d + 1], acc[:],
                        OP.add, OP.max,
                    )
                # ship {E127} and {D126, D127} one partition down into the
                # buffer consumed at d+2; Pool applies the tb==0 boundary NEG.
                rn_ = rings[(d + 2) % 3]
                nc.vector.stream_shuffle(rn_[:, 0:1], et[:, 127:128], shuf_mask)
                nc.vector.stream_shuffle(rn_[:, 1:3], rc[:, 129:131], shuf_mask)
                nc.gpsimd.tensor_scalar_min(rn_[:, 0:3], rn_[:, 0:3], bnc[:])

            # ---- phase A: lse exp-sums + g gather via transpose + one-hot
            # matmul, interleaved with phase-B diagonal emission so the DP's
            # g_t loads don't queue behind every logit load (sync queue FIFO)
            # and the wavefront starts as soon as its rows exist.
            lT_all = {}
            emitted = 0

            def emit_diags(upto):
                nonlocal emitted
                while emitted <= min(upto, NDIAG - 1):
                    phase_b_diag(emitted)
                    emitted += 1

            def a1_tile(ex, tb):
                lt = lpool.tile([128, V], f32, tag="lt")
                nc.sync.dma_start(
                    out=lt[:], in_=logits_e[ex, tb * TBS : (tb + 1) * TBS, :]
                )
                esc = lpool.tile([128, V], f32, tag="esc")
                nc.scalar.activation(
                    esc[:], lt[:], AF.Exp,
                    accum_out=sgrids[ex][:, tb : tb + 1],
                )
                ltb = lpool.tile([128, V], bf16, tag="ltb")
                nc.scalar.copy(ltb[:], lt[:])
                # +1e30 on the BLANK logit of invalid frames: the one-hot's
                # blank row (-1) then makes g = -1e30 there, capping D at each
                # example's last valid frame (replaces the em mask / wide acc).
                nc.gpsimd.tensor_scalar_add(
                    ltb[:, 0:1], ltb[:, 0:1], embs[ex][:, tb : tb + 1]
                )
                lTs = []
                for vc in range(2):
                    pst = ppool.tile([128, TBS], bf16, tag="pst")
                    nc.tensor.transpose(
                        pst[:], ltb[:, vc * 128 : (vc + 1) * 128], ident[:]
                    )
                    lT = cpool.tile(
                        [128, TBS], bf16,
                        tag=f"lT_{ex}_{tb}_{vc}", name=f"lT_{ex}_{tb}_{vc}",
                    )
                    # DVE is mostly idle while the DP crawls behind A1, and Act
                    # paces A1 — keep A1's PSUM->SBUF copies on DVE.
                    nc.vector.tensor_copy(lT[:], pst[:])
                    lTs.append(lT)
                lT_all[(ex, tb)] = lTs
                g_ps = ppool.tile([128, TBS], f32, tag="g_ps")
                for vc in range(2):
                    nc.tensor.matmul(
                        g_ps[:],
                        ohs[ex][:, vc * U : vc * U + 128],
                        lTs[vc][:],
                        start=(vc == 0),
                        stop=(vc == 1),
                    )
                g_sb = tpool.tile([128, TBS], f32, tag="g_sb")
                nc.vector.tensor_copy(g_sb[:], g_ps[:])
                r0 = 2 * tb
                nc.gpsimd.dma_start(
                    out=g_dram[ex, r0 : r0 + 128, tb * TBS : (tb + 1) * TBS],
                    in_=g_sb[:],
                )

            def a2_tile(ex, tb):
                lTs = lT_all[(ex, tb)]
                g_ps = ppool.tile([128, TBS], f32, tag="g_ps")
                for vc in range(2):
                    nc.tensor.matmul(
                        g_ps[:],
                        ohs[ex][:, vc * U + 128 : vc * U + 256],
                        lTs[vc][:],
                        start=(vc == 0),
                        stop=(vc == 1),
                    )
                g_sb = tpool.tile([128, TBS], f32, tag="g_sb")
                nc.scalar.copy(g_sb[:], g_ps[:])
                r0 = 128 + 2 * tb
                nc.gpsimd.dma_start(
                    out=g_dram[ex, r0 : r0 + 128, tb * TBS : (tb + 1) * TBS],
                    in_=g_sb[:],
                )

            for tb in range(NTB):
                for ex in range(BSH):
                    a1_tile(ex, tb)
                emit_diags(2 * tb)
            emit_diags(127)
            for tb in range(NTB):
                for ex in range(BSH):
                    a2_tile(ex, tb)
            emit_diags(NDIAG - 1)

            # ---- final assembly ----
            nc.sync.dma_start(out=f_dram[:].unsqueeze(1), in_=acc[:])
            tc.strict_bb_all_engine_barrier()
            f16 = dpool.tile([BSH, NTB], f32)
            nc.sync.dma_start(
                out=f16[:], in_=f_dram[:].rearrange("(e k) -> e k", k=NTB)
            )
            fvec = dpool.tile([BSH, 1], f32)
            nc.vector.tensor_reduce(fvec[:], f16[:], mybir.AxisListType.X, OP.max)

            pbs = dpool.tile([1, BSH * NTB + 1], f32)
            for ex in range(BSH):
                lns = tpool.tile([128, NTB], f32, tag="lns")
                nc.scalar.activation(lns[:], sgrids[ex][:], AF.Ln)
                l0g = tpool.tile([128, NTB], f32, tag="l0g")
                nc.sync.dma_start(
                    out=l0g[:],
                    in_=logits_e[ex, :, 0:1].rearrange("(b p) o -> p (b o)", p=TBS),
                )
                pbmk = tpool.tile([128, NTB], f32, tag="pbmk")
                nc.sync.dma_start(out=pbmk[:], in_=pbm_e[ex])
                pbm = tpool.tile([128, NTB], f32, tag="pbm")
                nc.vector.tensor_sub(pbm[:], l0g[:], lns[:])
                nc.vector.tensor_mul(pbm[:], pbm[:], pbmk[:])
                ps_col = fpool.tile([1, NTB], f32, tag="ps_col")
                nc.tensor.matmul(ps_col[:], ones128[:], pbm[:], start=True, stop=True)
                nc.scalar.copy(pbs[:, ex * NTB : (ex + 1) * NTB], ps_col[:])

            fv_ps = fpool.tile([1, 1], f32, tag="fv_ps")
            nc.tensor.matmul(
                fv_ps[:], ones128[0:BSH, :], fvec[:], start=True, stop=True
            )
            nc.scalar.copy(pbs[:, BSH * NTB : BSH * NTB + 1], fv_ps[:])

            score = dpool.tile([1, 1], f32)
            nc.vector.tensor_reduce(
                score[:], pbs[:], mybir.AxisListType.X, OP.add
            )
            nc.sync.dma_start(out=out_e[:].unsqueeze(0), in_=score[:])

    nc.finalize()
    return nc


def _get_nc():
    if "nc" not in _cached:
        _cached["nc"] = _build()
    return _cached["nc"]


def _host_tables(targets, loglen, tgtlen):
    import ml_dtypes

    bf16 = ml_dtypes.bfloat16
    Bfull = targets.shape[0]
    vv = np.arange(V, dtype=np.int64).reshape(2, 128)
    oh = (targets[:, None, None, :] == vv[None, :, :, None]).astype(np.float32)
    oh[:, 0, 0, :] = -1.0
    oh = np.ascontiguousarray(
        oh.transpose(0, 2, 1, 3).reshape(Bfull, 128, 2 * U)
    ).astype(bf16)
    # per-(core-partition, diagonal) tables; partition p = ex*16 + tb
    ncores = Bfull // BSH
    exg = np.arange(Bfull)  # global example
    rn_g = np.zeros((Bfull, U), np.float32)
    rn_g[:, 1:] = np.where(targets[:, 1:] == targets[:, :-1], np.float32(NEG), 0.0)
    ln_g = np.where(
        np.arange(U)[None, :] == (tgtlen[:, None] - 1), 0.0, NEG
    ).astype(np.float32)
    tbv = np.arange(NTB)
    dv = np.arange(NDIAG)
    # u[p, d] = d - 2*tb(p)
    uu = dv[None, :] - 2 * tbv[:, None]  # [NTB, NDIAG]
    inr = (uu >= 0) & (uu < U)
    uc = np.clip(uu, 0, U - 1)
    rn = np.zeros((ncores, 128, NDIAG), np.float32)
    ln = np.full((ncores, 128, NDIAG), NEG, np.float32)
    zz = np.full((ncores, 128, NDIAG), NEG, np.float32)
    for c in range(ncores):
        for e in range(BSH):
            b = c * BSH + e
            p0 = e * NTB
            rn[c, p0 : p0 + NTB] = np.where(inr, rn_g[b][uc], 0.0)
            ln[c, p0 : p0 + NTB] = np.where(inr, ln_g[b][uc], NEG)
            zz[c, p0 : p0 + NTB] = np.where(uu == 0, 0.0, NEG)
    # per-partition clamp for the halo ship: min(x, bnc) forces -1e30 at the
    # tb==0 boundary lanes and passes everything else through.
    bnc = np.full((128, 1), 3.0e38, np.float32)
    bnc[::16, 0] = NEG
    tglob = np.arange(NTB)[None, None, :] * TBS + np.arange(TBS)[None, :, None]
    pbm = (tglob < loglen[:, None, None]).astype(np.float32)
    # emb[b, p, tb] = +1e30 for frames beyond T_b-1 (added to the blank
    # logit so g becomes -1e30 there), 0 otherwise.
    emb = (1.0 - pbm) * 1.0e30
    ident = np.eye(128, dtype=np.float32).astype(bf16)
    ones = np.ones((128, 1), np.float32)
    return oh, rn, ln, zz, bnc, emb, pbm, ident, ones


def _build_in_maps(np_inputs):
    logits = np.ascontiguousarray(
        np.asarray(np_inputs["logits"], dtype=np.float32)
    )
    targets = np.asarray(np_inputs["targets"], dtype=np.int64)
    loglen = np.asarray(np_inputs["logits_lengths"], dtype=np.int64)
    tgtlen = np.asarray(np_inputs["targets_lengths"], dtype=np.int64)
    oh, rn, ln, zz, bnc, emb, pbm, ident, ones = _host_tables(
        targets, loglen, tgtlen
    )
    in_maps = []
    for c in range(NCORES):
        sl = slice(c * BSH, (c + 1) * BSH)
        in_maps.append(
            {
                "logits": logits[sl],
                "oh": np.ascontiguousarray(oh[sl]),
                "ident": ident,
                "rn": rn[c],
                "ln": ln[c],
                "zz": zz[c],
                "bnc": bnc,
                "emb": np.ascontiguousarray(emb[sl]),
                "pbm": np.ascontiguousarray(pbm[sl]),
                "ones": ones,
            }
        )
    return in_maps


def kernel(logits, targets, logits_lengths, targets_lengths):
    loglen = np.asarray(logits_lengths, dtype=np.int64)
    in_maps = _build_in_maps(
        dict(
            logits=logits,
            targets=targets,
            logits_lengths=logits_lengths,
            targets_lengths=targets_lengths,
        )
    )
    _get_nc()
    results = _run_spmd(in_maps)
    total = sum(float(r["out"][0]) for r in results)
    count = float(np.minimum(loglen, T).sum())
    return np.float32(-total / count)


def _make_runner():
    """Build a cached jitted SPMD runner (mirrors run_bass_via_pjrt) so repeat
    executions don't re-trace; used for both kernel() and benchmarking."""
    import jax
    import numpy as _np
    import concourse.mybir as mybir
    from concourse import bass2jax
    from jax.sharding import Mesh, PartitionSpec, NamedSharding
    from jax.experimental.shard_map import shard_map

    if "runner" in _cached:
        return _cached["runner"]

    nc = _get_nc()
    bass2jax.install_neuronx_cc_hook()

    partition_name = (
        nc.partition_id_tensor.name if nc.partition_id_tensor else None
    )
    in_names, out_names, out_avals, zero_outs = [], [], [], []
    for alloc in nc.m.functions[0].allocations:
        if not isinstance(alloc, mybir.MemoryLocationSet):
            continue
        name = alloc.memorylocations[0].name
        if alloc.kind == "ExternalInput":
            if name != partition_name:
                in_names.append(name)
        elif alloc.kind == "ExternalOutput":
            out_names.append(name)
            shape = tuple(alloc.tensor_shape)
            dtype = mybir.dt.np(alloc.dtype)
            out_avals.append(jax.core.ShapedArray(shape, dtype))
            zero_outs.append(_np.zeros(shape, dtype))
    n_params = len(in_names)
    n_outs = len(out_avals)
    all_names = in_names + out_names
    if partition_name is not None:
        all_names = all_names + [partition_name]

    def _body(*args):
        operands = list(args)
        if partition_name is not None:
            operands.append(bass2jax.partition_id_tensor())
        outs = bass2jax._bass_exec_p.bind(
            *operands,
            out_avals=tuple(out_avals),
            in_names=tuple(all_names),
            out_names=tuple(out_names),
            lowering_input_output_aliases=(),
            sim_require_finite=True,
            sim_require_nnan=True,
            nc=nc,
        )
        return tuple(outs)

    devices = jax.devices()[:NCORES]
    mesh = Mesh(np.asarray(devices), ("core",))
    in_specs = (PartitionSpec("core"),) * (n_params + n_outs)
    out_specs = (PartitionSpec("core"),) * n_outs
    donate = tuple(range(n_params, n_params + n_outs))
    sharded = jax.jit(
        shard_map(_body, mesh=mesh, in_specs=in_specs, out_specs=out_specs,
                  check_rep=False),
        donate_argnums=donate,
        keep_unused=True,
    )
    sharding = NamedSharding(mesh, PartitionSpec("core"))
    runner = dict(
        fn=sharded, in_names=in_names, out_names=out_names,
        zero_outs=zero_outs, sharding=sharding, n_params=n_params,
    )
    _cached["runner"] = runner
    return runner


def _run_spmd(in_maps):
    import jax
    r = _make_runner()
    per_core = [[_np_asarray(m[nm]) for nm in r["in_names"]] for m in in_maps]
    concat_in = [
        np.concatenate([per_core[c][i] for c in range(NCORES)], axis=0)
        for i in range(len(r["in_names"]))
    ]
    concat_zeros = [
        np.zeros((NCORES * z.shape[0], *z.shape[1:]), z.dtype)
        for z in r["zero_outs"]
    ]
    outs = r["fn"](*concat_in, *concat_zeros)
    res = []
    for c in range(NCORES):
        d = {}
        for i, nm in enumerate(r["out_names"]):
            d[nm] = np.asarray(outs[i]).reshape(NCORES, -1)[c]
        res.append(d)
    return res


def _np_asarray(x):
    return np.asarray(x)



# revision 31
# speedup vs baseline: 1.1809x; 1.1809x over previous
"""AlignedTargetsLoss (CTC forced-alignment Viterbi loss) on 8 TRN2 NeuronCores.

Key algebraic reduction: the masked-mean NLL of the Viterbi-aligned path equals
-(best path score)/count, and the best path score decomposes as
    score_b = PB_b + D[L_b-1][T_b-1]
where PB_b = sum_{t<T_b} (logits[t,0] - lse[t])  (blank log-prob prefix) and
D/E is a row DP over labels u (intervals formulation of the CTC state graph):
    E[u][t] = g_u[t] + max(E[u][t-1], P_u[t]),   g_u[t] = logits[t,y_u]-logits[t,0]
    P_u[t]  = max(D[u-1][t-1] + repneg_u, D[u-1][t-2]),  repneg = -inf if y_u==y_{u-1}
    D[u][t] = max(D[u][t-1], E[u][t])
E-scan and D-scan each map to one hardware tensor_tensor_scan instruction.
No backtrace needed: the loss only needs the path score.

g is produced on-device via PE transpose + one-hot matmul (the one-hot also
bakes in the -logits[:,0] subtraction), staged through DRAM, and streamed back
row-by-row for the DP. Small index-derived tables (one-hots, masks) are
precomputed on host. Sharding: pure data parallelism, 8 examples per core; the
host sums the per-core partial scores and divides by the total frame count.

Perf notes (measured via NTFF device profiles): the DVE scan chain
stt->scanE->scanD (~1.3us/diagonal, no 16-bit speedup exists for scans) is
the critical resource, so everything else is kept off DVE during the DP:
the halo boundary clamp runs on Pool (tensor_scalar_min), and the final-frame
readout is folded into g itself (+1e30 on the blank logit of invalid frames
via the emb table) so the per-diagonal accumulator is a [128,1] stt instead
of [128,128] plus a separate em mask. Only the never-written g_dram band
(rows [0,32) and [256,286)) is zeroed, so the wavefront starts ~4x earlier.
Phase A must issue fully before the DP diagonals: merged emission puts DP
loads ahead of logit loads in the sync-queue FIFO and stalls the wavefront
(measured 771-850us vs 719us baseline).
"""

import os
import sys

sys.path.insert(0, "/opt/trn_rl_repo")

import numpy as np

B, T, V, U = 64, 2048, 256, 256
NCORES = 8
BSH = B // NCORES  # 8 examples per core
NTB = 16  # t-blocks of 128
TBS = T // NTB  # 128
NDIAG = U + 2 * (NTB - 1)  # 286 wavefront diagonals
NEG = -1.0e30

_cached = {}


def _build():
    import concourse.bass as bass
    import concourse.bacc as bacc
    import concourse.mybir as mybir
    from concourse.tile import TileContext

    f32 = mybir.dt.float32
    bf16 = mybir.dt.bfloat16
    AF = mybir.ActivationFunctionType
    OP = mybir.AluOpType

    nc = bacc.Bacc()

    logits_e = nc.declare_dram_parameter("logits", [BSH, T, V], f32, isOutput=False)
    oh_e = nc.declare_dram_parameter("oh", [BSH, 128, 2 * U], bf16, isOutput=False)
    ident_e = nc.declare_dram_parameter("ident", [128, 128], bf16, isOutput=False)
    rn_e = nc.declare_dram_parameter("rn", [128, NDIAG], f32, isOutput=False)
    ln_e = nc.declare_dram_parameter("ln", [128, NDIAG], f32, isOutput=False)
    zz_e = nc.declare_dram_parameter("zz", [128, NDIAG], f32, isOutput=False)
    bnc_e = nc.declare_dram_parameter("bnc", [128, 1], f32, isOutput=False)
    emb_e = nc.declare_dram_parameter("emb", [BSH, 128, NTB], f32, isOutput=False)
    pbm_e = nc.declare_dram_parameter("pbm", [BSH, 128, NTB], f32, isOutput=False)
    ones_e = nc.declare_dram_parameter("ones", [128, 1], f32, isOutput=False)
    out_e = nc.declare_dram_parameter("out", [1], f32, isOutput=True)

    with TileContext(nc) as tc:
        import contextlib

        ctx = contextlib.ExitStack()
        with ctx:
            dramp = ctx.enter_context(tc.tile_pool(name="dram", bufs=1, space="DRAM"))
            cpool = ctx.enter_context(tc.tile_pool(name="const", bufs=1))
            lpool = ctx.enter_context(tc.tile_pool(name="logit", bufs=3))
            tpool = ctx.enter_context(tc.tile_pool(name="tmp", bufs=3))
            ppool = ctx.enter_context(tc.tile_pool(name="psum", bufs=2, space="PSUM"))
            fpool = ctx.enter_context(tc.tile_pool(name="fin", bufs=1, space="PSUM"))
            gpool = ctx.enter_context(tc.tile_pool(name="grow", bufs=4))
            dpool = ctx.enter_context(tc.tile_pool(name="dp", bufs=1))

            g_dram = dramp.tile([BSH, NDIAG, T], f32)
            f_dram = dramp.tile([128], f32)

            # ---- constant tables from host ----
            rnTab = cpool.tile([128, NDIAG], f32)
            nc.sync.dma_start(out=rnTab[:], in_=rn_e[:])
            lnTab = cpool.tile([128, NDIAG], f32)
            nc.sync.dma_start(out=lnTab[:], in_=ln_e[:])
            zzTab = cpool.tile([128, NDIAG], f32)
            nc.sync.dma_start(out=zzTab[:], in_=zz_e[:])
            bnc = cpool.tile([128, 1], f32)
            nc.sync.dma_start(out=bnc[:], in_=bnc_e[:])
            ident = cpool.tile([128, 128], bf16)
            nc.sync.dma_start(out=ident[:], in_=ident_e[:])
            ones128 = cpool.tile([128, 1], f32)
            nc.sync.dma_start(out=ones128[:], in_=ones_e[:])
            ohs = []
            for ex in range(BSH):
                oh = cpool.tile([128, 2 * U], bf16, tag=f"oh{ex}", name=f"oh{ex}")
                nc.sync.dma_start(out=oh[:], in_=oh_e[ex])
                ohs.append(oh)
            embs = []
            for ex in range(BSH):
                eb = cpool.tile([128, NTB], f32, tag=f"eb{ex}", name=f"eb{ex}")
                nc.sync.dma_start(out=eb[:], in_=emb_e[ex])
                embs.append(eb)

            # zero-fill ONLY the never-written g_dram band: rows [0,32) and
            # [256,286) (everything else is covered by phase A stores).
            zrow = cpool.tile([128, T], f32)
            nc.vector.memset(zrow[:], 0.0)
            for lo, hi in ((0, 32), (256, NDIAG)):
                for ex in range(BSH):
                    nc.gpsimd.dma_start(
                        out=g_dram[ex, lo:hi, :], in_=zrow[0 : hi - lo, :]
                    )

            sgrids = []
            for ex in range(BSH):
                sg = cpool.tile([128, NTB], f32, tag=f"sg{ex}", name=f"sg{ex}")
                sgrids.append(sg)

            # ---- DP state (phase B wavefront over (row u, t-block) diagonals)
            # partition p = ex*16 + tb; tile (u, tb) processed at d = u + 2*tb.
            # ring[i]: col 0 = E-carry-in, cols 1:3 = D-halo guards (D[-2], D[-1]),
            # cols 3:131 = this row-tile's D values.
            rings = []
            for i in range(3):
                rg = dpool.tile([128, 131], f32, tag=f"ring{i}", name=f"ring{i}")
                rings.append(rg)
                nc.vector.memset(rg[:, 0:3], NEG)
                nc.vector.memset(rg[:, 3:131], 0.0)
            ets = []
            for i in range(2):
                et = dpool.tile([128, TBS], f32, tag=f"et{i}", name=f"et{i}")
                ets.append(et)
            pt = dpool.tile([128, TBS], f32)
            acc = dpool.tile([128, 1], f32)
            nc.vector.memset(acc[:], NEG)
            shuf_mask = [i if i % 16 == 0 else i - 1 for i in range(32)]

            def phase_b_diag(d):
                g_t = gpool.tile([128, TBS], f32, tag="g_t")
                nc.sync.dma_start(
                    out=g_t[:],
                    in_=g_dram[:, d, :].rearrange("e (k t) -> e k t", k=NTB),
                )
                rp = rings[(d + 2) % 3]   # prev-row buffer (written at d-1)
                rc = rings[d % 3]         # current buffer (written now)
                et = ets[d % 2]
                # P = max(max(Dprev<<1 + rn, Dprev<<2), z)
                nc.vector.scalar_tensor_tensor(
                    pt[:], rp[:, 2:130], rnTab[:, d : d + 1], rp[:, 1:129],
                    OP.add, OP.max,
                )
                if d <= 2 * (NTB - 1) and d % 2 == 0:
                    nc.vector.tensor_scalar_max(
                        pt[:], pt[:], zzTab[:, d : d + 1]
                    )
                # E scan; carry-in at ring col 0 (shipped from left tile at d-2)
                nc.vector.tensor_tensor_scan(
                    et[:], pt[:], g_t[:], rc[:, 0:1], OP.max, OP.add
                )
                # D scan; carry-in = D[-1] guard (col 2, shipped at d-2)
                nc.vector.tensor_tensor_scan(
                    rc[:, 3:131], et[:], et[:], rc[:, 2:3], OP.max, OP.max
                )
                if d >= 127:
                    # emb poisons g beyond each example's last valid frame, so
                    # D[:,127] == D at the final frame; ln selects u == L-1.
                    nc.vector.scalar_tensor_tensor(
                        acc[:], rc[:, 130:131], lnTab[:, d : d + 1], acc[:],
                        OP.add, OP.max,
                    )
                # ship {E127} and {D126, D127} one partition down into the
                # buffer consumed at d+2; Pool applies the tb==0 boundary NEG.
                rn_ = rings[(d + 2) % 3]
                nc.vector.stream_shuffle(rn_[:, 0:1], et[:, 127:128], shuf_mask)
                nc.vector.stream_shuffle(rn_[:, 1:3], rc[:, 129:131], shuf_mask)
                nc.gpsimd.tensor_scalar_min(rn_[:, 0:3], rn_[:, 0:3], bnc[:])

            # ---- phase A: lse exp-sums + g gather via transpose + one-hot
            # matmul, interleaved with phase-B diagonal emission so the DP's
            # g_t loads don't queue behind every logit load (sync queue FIFO)
            # and the wavefront starts as soon as its rows exist.
            lT_all = {}
            emitted = 0

            def emit_diags(upto):
                nonlocal emitted
                while emitted <= min(upto, NDIAG - 1):
                    phase_b_diag(emitted)
                    emitted += 1

            def a1_tile(ex, tb):
                lt = lpool.tile([128, V], f32, tag="lt")
                nc.sync.dma_start(
                    out=lt[:], in_=logits_e[ex, tb * TBS : (tb + 1) * TBS, :]
                )
                esc = lpool.tile([128, V], f32, tag="esc")
                nc.scalar.activation(
                    esc[:], lt[:], AF.Exp,
                    accum_out=sgrids[ex][:, tb : tb + 1],
                )
                ltb = lpool.tile([128, V], bf16, tag="ltb")
                nc.scalar.copy(ltb[:], lt[:])
                # +1e30 on the BLANK logit of invalid frames: the one-hot's
                # blank row (-1) then makes g = -1e30 there, capping D at each
                # example's last valid frame (replaces the em mask / wide acc).
                nc.gpsimd.tensor_scalar_add(
                    ltb[:, 0:1], ltb[:, 0:1], embs[ex][:, tb : tb + 1]
                )
                lTs = []
                for vc in range(2):
                    pst = ppool.tile([128, TBS], bf16, tag="pst")
                    nc.tensor.transpose(
                        pst[:], ltb[:, vc * 128 : (vc + 1) * 128], ident[:]
                    )
                    lT = cpool.tile(
                        [128, TBS], bf16,
                        tag=f"lT_{ex}_{tb}_{vc}", name=f"lT_{ex}_{tb}_{vc}",
                    )
                    # DVE is mostly idle while the DP crawls behind A1, and Act
                    # paces A1 — keep A1's PSUM->SBUF copies on DVE.
                    nc.vector.tensor_copy(lT[:], pst[:])
                    lTs.append(lT)
                lT_all[(ex, tb)] = lTs
                g_ps = ppool.tile([128, TBS], f32, tag="g_ps")
                for vc in range(2):
                    nc.tensor.matmul(
                        g_ps[:],
                        ohs[ex][:, vc * U : vc * U + 128],
                        lTs[vc][:],
                        start=(vc == 0),
                        stop=(vc == 1),
                    )
                g_sb = tpool.tile([128, TBS], f32, tag="g_sb")
                nc.scalar.copy(g_sb[:], g_ps[:])
                r0 = 2 * tb
                nc.gpsimd.dma_start(
                    out=g_dram[ex, r0 : r0 + 128, tb * TBS : (tb + 1) * TBS],
                    in_=g_sb[:],
                )

            def a2_tile(ex, tb):
                lTs = lT_all[(ex, tb)]
                g_ps = ppool.tile([128, TBS], f32, tag="g_ps")
                for vc in range(2):
                    nc.tensor.matmul(
                        g_ps[:],
                        ohs[ex][:, vc * U + 128 : vc * U + 256],
                        lTs[vc][:],
                        start=(vc == 0),
                        stop=(vc == 1),
                    )
                g_sb = tpool.tile([128, TBS], f32, tag="g_sb")
                nc.scalar.copy(g_sb[:], g_ps[:])
                r0 = 128 + 2 * tb
                nc.gpsimd.dma_start(
                    out=g_dram[ex, r0 : r0 + 128, tb * TBS : (tb + 1) * TBS],
                    in_=g_sb[:],
                )

            # baseline-proven program order: all of phase A issues before the
            # DP diagonals (merged emission couples the sync-queue FIFO and
            # stalls the wavefront behind in-flight A1 tiles — measured worse).
            for tb in range(NTB):
                for ex in range(BSH):
                    a1_tile(ex, tb)
            for tb in range(NTB):
                for ex in range(BSH):
                    a2_tile(ex, tb)
            emit_diags(NDIAG - 1)

            # ---- final assembly ----
            nc.sync.dma_start(out=f_dram[:].unsqueeze(1), in_=acc[:])
            tc.strict_bb_all_engine_barrier()
            f16 = dpool.tile([BSH, NTB], f32)
            nc.sync.dma_start(
                out=f16[:], in_=f_dram[:].rearrange("(e k) -> e k", k=NTB)
            )
            fvec = dpool.tile([BSH, 1], f32)
            nc.vector.tensor_reduce(fvec[:], f16[:], mybir.AxisListType.X, OP.max)

            pbs = dpool.tile([1, BSH * NTB + 1], f32)
            for ex in range(BSH):
                lns = tpool.tile([128, NTB], f32, tag="lns")
                nc.scalar.activation(lns[:], sgrids[ex][:], AF.Ln)
                l0g = tpool.tile([128, NTB], f32, tag="l0g")
                nc.sync.dma_start(
                    out=l0g[:],
                    in_=logits_e[ex, :, 0:1].rearrange("(b p) o -> p (b o)", p=TBS),
                )
                pbmk = tpool.tile([128, NTB], f32, tag="pbmk")
                nc.sync.dma_start(out=pbmk[:], in_=pbm_e[ex])
                pbm = tpool.tile([128, NTB], f32, tag="pbm")
                nc.vector.tensor_sub(pbm[:], l0g[:], lns[:])
                nc.vector.tensor_mul(pbm[:], pbm[:], pbmk[:])
                ps_col = fpool.tile([1, NTB], f32, tag="ps_col")
                nc.tensor.matmul(ps_col[:], ones128[:], pbm[:], start=True, stop=True)
                nc.scalar.copy(pbs[:, ex * NTB : (ex + 1) * NTB], ps_col[:])

            fv_ps = fpool.tile([1, 1], f32, tag="fv_ps")
            nc.tensor.matmul(
                fv_ps[:], ones128[0:BSH, :], fvec[:], start=True, stop=True
            )
            nc.scalar.copy(pbs[:, BSH * NTB : BSH * NTB + 1], fv_ps[:])

            score = dpool.tile([1, 1], f32)
            nc.vector.tensor_reduce(
                score[:], pbs[:], mybir.AxisListType.X, OP.add
            )
            nc.sync.dma_start(out=out_e[:].unsqueeze(0), in_=score[:])

    nc.finalize()
    return nc


def _get_nc():
    if "nc" not in _cached:
        _cached["nc"] = _build()
    return _cached["nc"]


def _host_tables(targets, loglen, tgtlen):
    import ml_dtypes

    bf16 = ml_dtypes.bfloat16
    Bfull = targets.shape[0]
    vv = np.arange(V, dtype=np.int64).reshape(2, 128)
    oh = (targets[:, None, None, :] == vv[None, :, :, None]).astype(np.float32)
    oh[:, 0, 0, :] = -1.0
    oh = np.ascontiguousarray(
        oh.transpose(0, 2, 1, 3).reshape(Bfull, 128, 2 * U)
    ).astype(bf16)
    # per-(core-partition, diagonal) tables; partition p = ex*16 + tb
    ncores = Bfull // BSH
    exg = np.arange(Bfull)  # global example
    rn_g = np.zeros((Bfull, U), np.float32)
    rn_g[:, 1:] = np.where(targets[:, 1:] == targets[:, :-1], np.float32(NEG), 0.0)
    ln_g = np.where(
        np.arange(U)[None, :] == (tgtlen[:, None] - 1), 0.0, NEG
    ).astype(np.float32)
    tbv = np.arange(NTB)
    dv = np.arange(NDIAG)
    # u[p, d] = d - 2*tb(p)
    uu = dv[None, :] - 2 * tbv[:, None]  # [NTB, NDIAG]
    inr = (uu >= 0) & (uu < U)
    uc = np.clip(uu, 0, U - 1)
    rn = np.zeros((ncores, 128, NDIAG), np.float32)
    ln = np.full((ncores, 128, NDIAG), NEG, np.float32)
    zz = np.full((ncores, 128, NDIAG), NEG, np.float32)
    for c in range(ncores):
        for e in range(BSH):
            b = c * BSH + e
            p0 = e * NTB
            rn[c, p0 : p0 + NTB] = np.where(inr, rn_g[b][uc], 0.0)
            ln[c, p0 : p0 + NTB] = np.where(inr, ln_g[b][uc], NEG)
            zz[c, p0 : p0 + NTB] = np.where(uu == 0, 0.0, NEG)
    # per-partition clamp for the halo ship: min(x, bnc) forces -1e30 at the
    # tb==0 boundary lanes and passes everything else through.
    bnc = np.full((128, 1), 3.0e38, np.float32)
    bnc[::16, 0] = NEG
    tglob = np.arange(NTB)[None, None, :] * TBS + np.arange(TBS)[None, :, None]
    pbm = (tglob < loglen[:, None, None]).astype(np.float32)
    # emb[b, p, tb] = +1e30 for frames beyond T_b-1 (added to the blank
    # logit so g becomes -1e30 there), 0 otherwise.
    emb = (1.0 - pbm) * 1.0e30
    ident = np.eye(128, dtype=np.float32).astype(bf16)
    ones = np.ones((128, 1), np.float32)
    return oh, rn, ln, zz, bnc, emb, pbm, ident, ones


def _build_in_maps(np_inputs):
    logits = np.ascontiguousarray(
        np.asarray(np_inputs["logits"], dtype=np.float32)
    )
    targets = np.asarray(np_inputs["targets"], dtype=np.int64)
    loglen = np.asarray(np_inputs["logits_lengths"], dtype=np.int64)
    tgtlen = np.asarray(np_inputs["targets_lengths"], dtype=np.int64)
    oh, rn, ln, zz, bnc, emb, pbm, ident, ones = _host_tables(
        targets, loglen, tgtlen
    )
    in_maps = []
    for c in range(NCORES):
        sl = slice(c * BSH, (c + 1) * BSH)
        in_maps.append(
            {
                "logits": logits[sl],
                "oh": np.ascontiguousarray(oh[sl]),
                "ident": ident,
                "rn": rn[c],
                "ln": ln[c],
                "zz": zz[c],
                "bnc": bnc,
                "emb": np.ascontiguousarray(emb[sl]),
                "pbm": np.ascontiguousarray(pbm[sl]),
                "ones": ones,
            }
        )
    return in_maps


def kernel(logits, targets, logits_lengths, targets_lengths):
    loglen = np.asarray(logits_lengths, dtype=np.int64)
    in_maps = _build_in_maps(
        dict(
            logits=logits,
            targets=targets,
            logits_lengths=logits_lengths,
            targets_lengths=targets_lengths,
        )
    )
    _get_nc()
    results = _run_spmd(in_maps)
    total = sum(float(r["out"][0]) for r in results)
    count = float(np.minimum(loglen, T).sum())
    return np.float32(-total / count)


def _make_runner():
    """Build a cached jitted SPMD runner (mirrors run_bass_via_pjrt) so repeat
    executions don't re-trace; used for both kernel() and benchmarking."""
    import jax
    import numpy as _np
    import concourse.mybir as mybir
    from concourse import bass2jax
    from jax.sharding import Mesh, PartitionSpec, NamedSharding
    from jax.experimental.shard_map import shard_map

    if "runner" in _cached:
        return _cached["runner"]

    nc = _get_nc()
    bass2jax.install_neuronx_cc_hook()

    partition_name = (
        nc.partition_id_tensor.name if nc.partition_id_tensor else None
    )
    in_names, out_names, out_avals, zero_outs = [], [], [], []
    for alloc in nc.m.functions[0].allocations:
        if not isinstance(alloc, mybir.MemoryLocationSet):
            continue
        name = alloc.memorylocations[0].name
        if alloc.kind == "ExternalInput":
            if name != partition_name:
                in_names.append(name)
        elif alloc.kind == "ExternalOutput":
            out_names.append(name)
            shape = tuple(alloc.tensor_shape)
            dtype = mybir.dt.np(alloc.dtype)
            out_avals.append(jax.core.ShapedArray(shape, dtype))
            zero_outs.append(_np.zeros(shape, dtype))
    n_params = len(in_names)
    n_outs = len(out_avals)
    all_names = in_names + out_names
    if partition_name is not None:
        all_names = all_names + [partition_name]

    def _body(*args):
        operands = list(args)
        if partition_name is not None:
            operands.append(bass2jax.partition_id_tensor())
        outs = bass2jax._bass_exec_p.bind(
            *operands,
            out_avals=tuple(out_avals),
            in_names=tuple(all_names),
            out_names=tuple(out_names),
            lowering_input_output_aliases=(),
            sim_require_finite=True,
            sim_require_nnan=True,
            nc=nc,
        )
        return tuple(outs)

    devices = jax.devices()[:NCORES]
    mesh = Mesh(np.asarray(devices), ("core",))
    in_specs = (PartitionSpec("core"),) * (n_params + n_outs)
    out_specs = (PartitionSpec("core"),) * n_outs
    donate = tuple(range(n_params, n_params + n_outs))
    sharded = jax.jit(
        shard_map(_body, mesh=mesh, in_specs=in_specs, out_specs=out_specs,
                  check_rep=False),
        donate_argnums=donate,
        keep_unused=True,
    )
    sharding = NamedSharding(mesh, PartitionSpec("core"))
    runner = dict(
        fn=sharded, in_names=in_names, out_names=out_names,
        zero_outs=zero_outs, sharding=sharding, n_params=n_params,
    )
    _cached["runner"] = runner
    return runner


def _run_spmd(in_maps):
    import jax
    r = _make_runner()
    per_core = [[_np_asarray(m[nm]) for nm in r["in_names"]] for m in in_maps]
    concat_in = [
        np.concatenate([per_core[c][i] for c in range(NCORES)], axis=0)
        for i in range(len(r["in_names"]))
    ]
    concat_zeros = [
        np.zeros((NCORES * z.shape[0], *z.shape[1:]), z.dtype)
        for z in r["zero_outs"]
    ]
    outs = r["fn"](*concat_in, *concat_zeros)
    res = []
    for c in range(NCORES):
        d = {}
        for i, nm in enumerate(r["out_names"]):
            d[nm] = np.asarray(outs[i]).reshape(NCORES, -1)[c]
        res.append(d)
    return res


def _np_asarray(x):
    return np.asarray(x)



# revision 32
# speedup vs baseline: 1.2065x; 1.0217x over previous
"""AlignedTargetsLoss (CTC forced-alignment Viterbi loss) on 8 TRN2 NeuronCores.

Key algebraic reduction: the masked-mean NLL of the Viterbi-aligned path equals
-(best path score)/count, and the best path score decomposes as
    score_b = PB_b + D[L_b-1][T_b-1]
where PB_b = sum_{t<T_b} (logits[t,0] - lse[t])  (blank log-prob prefix) and
D/E is a row DP over labels u (intervals formulation of the CTC state graph):
    E[u][t] = g_u[t] + max(E[u][t-1], P_u[t]),   g_u[t] = logits[t,y_u]-logits[t,0]
    P_u[t]  = max(D[u-1][t-1] + repneg_u, D[u-1][t-2]),  repneg = -inf if y_u==y_{u-1}
    D[u][t] = max(D[u][t-1], E[u][t])
E-scan and D-scan each map to one hardware tensor_tensor_scan instruction.
No backtrace needed: the loss only needs the path score.

g is produced on-device via PE transpose + one-hot matmul (the one-hot also
bakes in the -logits[:,0] subtraction), staged through DRAM, and streamed back
row-by-row for the DP. Small index-derived tables (one-hots, masks) are
precomputed on host. Sharding: pure data parallelism, 8 examples per core; the
host sums the per-core partial scores and divides by the total frame count.

Perf notes (measured via NTFF device profiles): the DVE scan chain
stt->scanE->scanD (~1.3us/diagonal, no 16-bit speedup exists for scans) is
the critical resource, so everything else is kept off DVE during the DP:
the halo boundary clamp runs on Pool (tensor_scalar_min), and the final-frame
readout is folded into g itself (+1e30 on the blank logit of invalid frames
via the emb table) so the per-diagonal accumulator is a [128,1] stt instead
of [128,128] plus a separate em mask. Only the never-written g_dram band
(rows [0,32) and [256,286)) is zeroed, so the wavefront starts ~4x earlier.
Phase A must issue fully before the DP diagonals: merged emission puts DP
loads ahead of logit loads in the sync-queue FIFO and stalls the wavefront
(measured 771-850us vs 719us baseline).
"""

import os
import sys

sys.path.insert(0, "/opt/trn_rl_repo")

import numpy as np

B, T, V, U = 64, 2048, 256, 256
NCORES = 8
BSH = B // NCORES  # 8 examples per core
NTB = 16  # t-blocks of 128
TBS = T // NTB  # 128
NDIAG = U + 2 * (NTB - 1)  # 286 wavefront diagonals
NEG = -1.0e30

_cached = {}


def _build():
    import concourse.bass as bass
    import concourse.bacc as bacc
    import concourse.mybir as mybir
    from concourse.tile import TileContext

    f32 = mybir.dt.float32
    bf16 = mybir.dt.bfloat16
    AF = mybir.ActivationFunctionType
    OP = mybir.AluOpType

    nc = bacc.Bacc()

    logits_e = nc.declare_dram_parameter("logits", [BSH, T, V], f32, isOutput=False)
    oh_e = nc.declare_dram_parameter("oh", [BSH, 128, 2 * U], bf16, isOutput=False)
    ident_e = nc.declare_dram_parameter("ident", [128, 128], bf16, isOutput=False)
    rn_e = nc.declare_dram_parameter("rn", [128, NDIAG], f32, isOutput=False)
    ln_e = nc.declare_dram_parameter("ln", [128, NDIAG], f32, isOutput=False)
    zz_e = nc.declare_dram_parameter("zz", [128, NDIAG], f32, isOutput=False)
    bnc_e = nc.declare_dram_parameter("bnc", [128, 1], f32, isOutput=False)
    emb_e = nc.declare_dram_parameter("emb", [BSH, 128, NTB], f32, isOutput=False)
    pbm_e = nc.declare_dram_parameter("pbm", [BSH, 128, NTB], f32, isOutput=False)
    ones_e = nc.declare_dram_parameter("ones", [128, 1], f32, isOutput=False)
    out_e = nc.declare_dram_parameter("out", [1], f32, isOutput=True)

    with TileContext(nc) as tc:
        import contextlib

        ctx = contextlib.ExitStack()
        with ctx:
            dramp = ctx.enter_context(tc.tile_pool(name="dram", bufs=1, space="DRAM"))
            cpool = ctx.enter_context(tc.tile_pool(name="const", bufs=1))
            lpool = ctx.enter_context(tc.tile_pool(name="logit", bufs=3))
            tpool = ctx.enter_context(tc.tile_pool(name="tmp", bufs=3))
            ppool = ctx.enter_context(tc.tile_pool(name="psum", bufs=2, space="PSUM"))
            fpool = ctx.enter_context(tc.tile_pool(name="fin", bufs=1, space="PSUM"))
            gpool = ctx.enter_context(tc.tile_pool(name="grow", bufs=4))
            dpool = ctx.enter_context(tc.tile_pool(name="dp", bufs=1))

            g_dram = dramp.tile([BSH, NDIAG, T], f32)
            f_dram = dramp.tile([128], f32)

            # ---- constant tables from host ----
            rnTab = cpool.tile([128, NDIAG], f32)
            nc.sync.dma_start(out=rnTab[:], in_=rn_e[:])
            lnTab = cpool.tile([128, NDIAG], f32)
            nc.sync.dma_start(out=lnTab[:], in_=ln_e[:])
            zzTab = cpool.tile([128, NDIAG], f32)
            nc.sync.dma_start(out=zzTab[:], in_=zz_e[:])
            bnc = cpool.tile([128, 1], f32)
            nc.sync.dma_start(out=bnc[:], in_=bnc_e[:])
            ident = cpool.tile([128, 128], bf16)
            nc.sync.dma_start(out=ident[:], in_=ident_e[:])
            ones128 = cpool.tile([128, 1], f32)
            nc.sync.dma_start(out=ones128[:], in_=ones_e[:])
            ohs = []
            for ex in range(BSH):
                oh = cpool.tile([128, 2 * U], bf16, tag=f"oh{ex}", name=f"oh{ex}")
                nc.sync.dma_start(out=oh[:], in_=oh_e[ex])
                ohs.append(oh)
            embs = []
            for ex in range(BSH):
                eb = cpool.tile([128, NTB], f32, tag=f"eb{ex}", name=f"eb{ex}")
                nc.sync.dma_start(out=eb[:], in_=emb_e[ex])
                embs.append(eb)

            # zero-fill ONLY the never-written g_dram band: rows [0,32) and
            # [256,286) (everything else is covered by phase A stores).
            zrow = cpool.tile([128, T], f32)
            nc.vector.memset(zrow[:], 0.0)
            for lo, hi in ((0, 32), (256, NDIAG)):
                for ex in range(BSH):
                    nc.gpsimd.dma_start(
                        out=g_dram[ex, lo:hi, :], in_=zrow[0 : hi - lo, :]
                    )

            sgrids = []
            for ex in range(BSH):
                sg = cpool.tile([128, NTB], f32, tag=f"sg{ex}", name=f"sg{ex}")
                sgrids.append(sg)

            # ---- DP state (phase B wavefront over (row u, t-block) diagonals)
            # partition p = ex*16 + tb; tile (u, tb) processed at d = u + 2*tb.
            # ring[i]: col 0 = E-carry-in, cols 1:3 = D-halo guards (D[-2], D[-1]),
            # cols 3:131 = this row-tile's D values.
            rings = []
            for i in range(3):
                rg = dpool.tile([128, 131], f32, tag=f"ring{i}", name=f"ring{i}")
                rings.append(rg)
                nc.vector.memset(rg[:, 0:3], NEG)
                nc.vector.memset(rg[:, 3:131], 0.0)
            ets = []
            for i in range(2):
                et = dpool.tile([128, TBS], f32, tag=f"et{i}", name=f"et{i}")
                ets.append(et)
            pt = dpool.tile([128, TBS], f32)
            acc = dpool.tile([128, 1], f32)
            nc.vector.memset(acc[:], NEG)
            shuf_mask = [i if i % 16 == 0 else i - 1 for i in range(32)]

            def phase_b_diag(d):
                g_t = gpool.tile([128, TBS], f32, tag="g_t")
                nc.sync.dma_start(
                    out=g_t[:],
                    in_=g_dram[:, d, :].rearrange("e (k t) -> e k t", k=NTB),
                )
                rp = rings[(d + 2) % 3]   # prev-row buffer (written at d-1)
                rc = rings[d % 3]         # current buffer (written now)
                et = ets[d % 2]
                # P = max(max(Dprev<<1 + rn, Dprev<<2), z)
                nc.vector.scalar_tensor_tensor(
                    pt[:], rp[:, 2:130], rnTab[:, d : d + 1], rp[:, 1:129],
                    OP.add, OP.max,
                )
                if d <= 2 * (NTB - 1) and d % 2 == 0:
                    nc.vector.tensor_scalar_max(
                        pt[:], pt[:], zzTab[:, d : d + 1]
                    )
                # E scan; carry-in at ring col 0 (shipped from left tile at d-2)
                nc.vector.tensor_tensor_scan(
                    et[:], pt[:], g_t[:], rc[:, 0:1], OP.max, OP.add
                )
                # D scan; carry-in = D[-1] guard (col 2, shipped at d-2)
                nc.vector.tensor_tensor_scan(
                    rc[:, 3:131], et[:], et[:], rc[:, 2:3], OP.max, OP.max
                )
                if d >= 127:
                    # emb poisons g beyond each example's last valid frame, so
                    # D[:,127] == D at the final frame; ln selects u == L-1.
                    nc.vector.scalar_tensor_tensor(
                        acc[:], rc[:, 130:131], lnTab[:, d : d + 1], acc[:],
                        OP.add, OP.max,
                    )
                # ship {E127} and {D126, D127} one partition down into the
                # buffer consumed at d+2; Pool applies the tb==0 boundary NEG.
                rn_ = rings[(d + 2) % 3]
                nc.vector.stream_shuffle(rn_[:, 0:1], et[:, 127:128], shuf_mask)
                nc.vector.stream_shuffle(rn_[:, 1:3], rc[:, 129:131], shuf_mask)
                nc.gpsimd.tensor_scalar_min(rn_[:, 0:3], rn_[:, 0:3], bnc[:])

            # ---- phase A: lse exp-sums + g gather via transpose + one-hot
            # matmul, interleaved with phase-B diagonal emission so the DP's
            # g_t loads don't queue behind every logit load (sync queue FIFO)
            # and the wavefront starts as soon as its rows exist.
            lT_all = {}
            emitted = 0

            def emit_diags(upto):
                nonlocal emitted
                while emitted <= min(upto, NDIAG - 1):
                    phase_b_diag(emitted)
                    emitted += 1

            def a1_tile(ex, tb):
                lt = lpool.tile([128, V], f32, tag="lt")
                nc.sync.dma_start(
                    out=lt[:], in_=logits_e[ex, tb * TBS : (tb + 1) * TBS, :]
                )
                esc = lpool.tile([128, V], f32, tag="esc")
                nc.scalar.activation(
                    esc[:], lt[:], AF.Exp,
                    accum_out=sgrids[ex][:, tb : tb + 1],
                )
                ltb = lpool.tile([128, V], bf16, tag="ltb")
                nc.scalar.copy(ltb[:], lt[:])
                # +1e30 on the BLANK logit of invalid frames: the one-hot's
                # blank row (-1) then makes g = -1e30 there, capping D at each
                # example's last valid frame (replaces the em mask / wide acc).
                nc.gpsimd.tensor_scalar_add(
                    ltb[:, 0:1], ltb[:, 0:1], embs[ex][:, tb : tb + 1]
                )
                lTs = []
                for vc in range(2):
                    pst = ppool.tile([128, TBS], bf16, tag="pst")
                    nc.tensor.transpose(
                        pst[:], ltb[:, vc * 128 : (vc + 1) * 128], ident[:]
                    )
                    lT = cpool.tile(
                        [128, TBS], bf16,
                        tag=f"lT_{ex}_{tb}_{vc}", name=f"lT_{ex}_{tb}_{vc}",
                    )
                    # DVE is mostly idle while the DP crawls behind A1, and Act
                    # paces A1 — keep A1's PSUM->SBUF copies on DVE.
                    nc.vector.tensor_copy(lT[:], pst[:])
                    lTs.append(lT)
                lT_all[(ex, tb)] = lTs
                g_ps = ppool.tile([128, TBS], f32, tag="g_ps")
                for vc in range(2):
                    nc.tensor.matmul(
                        g_ps[:],
                        ohs[ex][:, vc * U : vc * U + 128],
                        lTs[vc][:],
                        start=(vc == 0),
                        stop=(vc == 1),
                    )
                g_sb = tpool.tile([128, TBS], f32, tag="g_sb")
                nc.vector.tensor_copy(g_sb[:], g_ps[:])
                r0 = 2 * tb
                nc.gpsimd.dma_start(
                    out=g_dram[ex, r0 : r0 + 128, tb * TBS : (tb + 1) * TBS],
                    in_=g_sb[:],
                )

            def a2_tile(ex, tb):
                lTs = lT_all[(ex, tb)]
                g_ps = ppool.tile([128, TBS], f32, tag="g_ps")
                for vc in range(2):
                    nc.tensor.matmul(
                        g_ps[:],
                        ohs[ex][:, vc * U + 128 : vc * U + 256],
                        lTs[vc][:],
                        start=(vc == 0),
                        stop=(vc == 1),
                    )
                g_sb = tpool.tile([128, TBS], f32, tag="g_sb")
                nc.scalar.copy(g_sb[:], g_ps[:])
                r0 = 128 + 2 * tb
                nc.gpsimd.dma_start(
                    out=g_dram[ex, r0 : r0 + 128, tb * TBS : (tb + 1) * TBS],
                    in_=g_sb[:],
                )

            # baseline-proven program order: all of phase A issues before the
            # DP diagonals (merged emission couples the sync-queue FIFO and
            # stalls the wavefront behind in-flight A1 tiles — measured worse).
            for tb in range(NTB):
                for ex in range(BSH):
                    a1_tile(ex, tb)
            for tb in range(NTB):
                for ex in range(BSH):
                    a2_tile(ex, tb)

            # PB (blank-prefix) terms only need sgrids (A1 exp sums) and the
            # raw logits -- issue before the DP diagonals so they overlap it.
            pbs = dpool.tile([1, BSH * NTB + 1], f32)
            for ex in range(BSH):
                lns = tpool.tile([128, NTB], f32, tag="lns")
                nc.scalar.activation(lns[:], sgrids[ex][:], AF.Ln)
                l0g = tpool.tile([128, NTB], f32, tag="l0g")
                nc.sync.dma_start(
                    out=l0g[:],
                    in_=logits_e[ex, :, 0:1].rearrange("(b p) o -> p (b o)", p=TBS),
                )
                pbmk = tpool.tile([128, NTB], f32, tag="pbmk")
                nc.sync.dma_start(out=pbmk[:], in_=pbm_e[ex])
                pbm = tpool.tile([128, NTB], f32, tag="pbm")
                nc.vector.tensor_sub(pbm[:], l0g[:], lns[:])
                nc.vector.tensor_mul(pbm[:], pbm[:], pbmk[:])
                ps_col = fpool.tile([1, NTB], f32, tag="ps_col")
                nc.tensor.matmul(ps_col[:], ones128[:], pbm[:], start=True, stop=True)
                nc.scalar.copy(pbs[:, ex * NTB : (ex + 1) * NTB], ps_col[:])

            emit_diags(NDIAG - 1)

            # ---- final assembly ----
            nc.sync.dma_start(out=f_dram[:].unsqueeze(1), in_=acc[:])
            tc.strict_bb_all_engine_barrier()
            f16 = dpool.tile([BSH, NTB], f32)
            nc.sync.dma_start(
                out=f16[:], in_=f_dram[:].rearrange("(e k) -> e k", k=NTB)
            )
            fvec = dpool.tile([BSH, 1], f32)
            nc.vector.tensor_reduce(fvec[:], f16[:], mybir.AxisListType.X, OP.max)

            fv_ps = fpool.tile([1, 1], f32, tag="fv_ps")
            nc.tensor.matmul(
                fv_ps[:], ones128[0:BSH, :], fvec[:], start=True, stop=True
            )
            nc.scalar.copy(pbs[:, BSH * NTB : BSH * NTB + 1], fv_ps[:])

            score = dpool.tile([1, 1], f32)
            nc.vector.tensor_reduce(
                score[:], pbs[:], mybir.AxisListType.X, OP.add
            )
            nc.sync.dma_start(out=out_e[:].unsqueeze(0), in_=score[:])

    nc.finalize()
    return nc


def _get_nc():
    if "nc" not in _cached:
        _cached["nc"] = _build()
    return _cached["nc"]


def _host_tables(targets, loglen, tgtlen):
    import ml_dtypes

    bf16 = ml_dtypes.bfloat16
    Bfull = targets.shape[0]
    vv = np.arange(V, dtype=np.int64).reshape(2, 128)
    oh = (targets[:, None, None, :] == vv[None, :, :, None]).astype(np.float32)
    oh[:, 0, 0, :] = -1.0
    oh = np.ascontiguousarray(
        oh.transpose(0, 2, 1, 3).reshape(Bfull, 128, 2 * U)
    ).astype(bf16)
    # per-(core-partition, diagonal) tables; partition p = ex*16 + tb
    ncores = Bfull // BSH
    exg = np.arange(Bfull)  # global example
    rn_g = np.zeros((Bfull, U), np.float32)
    rn_g[:, 1:] = np.where(targets[:, 1:] == targets[:, :-1], np.float32(NEG), 0.0)
    ln_g = np.where(
        np.arange(U)[None, :] == (tgtlen[:, None] - 1), 0.0, NEG
    ).astype(np.float32)
    tbv = np.arange(NTB)
    dv = np.arange(NDIAG)
    # u[p, d] = d - 2*tb(p)
    uu = dv[None, :] - 2 * tbv[:, None]  # [NTB, NDIAG]
    inr = (uu >= 0) & (uu < U)
    uc = np.clip(uu, 0, U - 1)
    rn = np.zeros((ncores, 128, NDIAG), np.float32)
    ln = np.full((ncores, 128, NDIAG), NEG, np.float32)
    zz = np.full((ncores, 128, NDIAG), NEG, np.float32)
    for c in range(ncores):
        for e in range(BSH):
            b = c * BSH + e
            p0 = e * NTB
            rn[c, p0 : p0 + NTB] = np.where(inr, rn_g[b][uc], 0.0)
            ln[c, p0 : p0 + NTB] = np.where(inr, ln_g[b][uc], NEG)
            zz[c, p0 : p0 + NTB] = np.where(uu == 0, 0.0, NEG)
    # per-partition clamp for the halo ship: min(x, bnc) forces -1e30 at the
    # tb==0 boundary lanes and passes everything else through.
    bnc = np.full((128, 1), 3.0e38, np.float32)
    bnc[::16, 0] = NEG
    tglob = np.arange(NTB)[None, None, :] * TBS + np.arange(TBS)[None, :, None]
    pbm = (tglob < loglen[:, None, None]).astype(np.float32)
    # emb[b, p, tb] = +1e30 for frames beyond T_b-1 (added to the blank
    # logit so g becomes -1e30 there), 0 otherwise.
    emb = (1.0 - pbm) * 1.0e30
    ident = np.eye(128, dtype=np.float32).astype(bf16)
    ones = np.ones((128, 1), np.float32)
    return oh, rn, ln, zz, bnc, emb, pbm, ident, ones


def _build_in_maps(np_inputs):
    logits = np.ascontiguousarray(
        np.asarray(np_inputs["logits"], dtype=np.float32)
    )
    targets = np.asarray(np_inputs["targets"], dtype=np.int64)
    loglen = np.asarray(np_inputs["logits_lengths"], dtype=np.int64)
    tgtlen = np.asarray(np_inputs["targets_lengths"], dtype=np.int64)
    oh, rn, ln, zz, bnc, emb, pbm, ident, ones = _host_tables(
        targets, loglen, tgtlen
    )
    in_maps = []
    for c in range(NCORES):
        sl = slice(c * BSH, (c + 1) * BSH)
        in_maps.append(
            {
                "logits": logits[sl],
                "oh": np.ascontiguousarray(oh[sl]),
                "ident": ident,
                "rn": rn[c],
                "ln": ln[c],
                "zz": zz[c],
                "bnc": bnc,
                "emb": np.ascontiguousarray(emb[sl]),
                "pbm": np.ascontiguousarray(pbm[sl]),
                "ones": ones,
            }
        )
    return in_maps


def kernel(logits, targets, logits_lengths, targets_lengths):
    loglen = np.asarray(logits_lengths, dtype=np.int64)
    in_maps = _build_in_maps(
        dict(
            logits=logits,
            targets=targets,
            logits_lengths=logits_lengths,
            targets_lengths=targets_lengths,
        )
    )
    _get_nc()
    results = _run_spmd(in_maps)
    total = sum(float(r["out"][0]) for r in results)
    count = float(np.minimum(loglen, T).sum())
    return np.float32(-total / count)


def _make_runner():
    """Build a cached jitted SPMD runner (mirrors run_bass_via_pjrt) so repeat
    executions don't re-trace; used for both kernel() and benchmarking."""
    import jax
    import numpy as _np
    import concourse.mybir as mybir
    from concourse import bass2jax
    from jax.sharding import Mesh, PartitionSpec, NamedSharding
    from jax.experimental.shard_map import shard_map

    if "runner" in _cached:
        return _cached["runner"]

    nc = _get_nc()
    bass2jax.install_neuronx_cc_hook()

    partition_name = (
        nc.partition_id_tensor.name if nc.partition_id_tensor else None
    )
    in_names, out_names, out_avals, zero_outs = [], [], [], []
    for alloc in nc.m.functions[0].allocations:
        if not isinstance(alloc, mybir.MemoryLocationSet):
            continue
        name = alloc.memorylocations[0].name
        if alloc.kind == "ExternalInput":
            if name != partition_name:
                in_names.append(name)
        elif alloc.kind == "ExternalOutput":
            out_names.append(name)
            shape = tuple(alloc.tensor_shape)
            dtype = mybir.dt.np(alloc.dtype)
            out_avals.append(jax.core.ShapedArray(shape, dtype))
            zero_outs.append(_np.zeros(shape, dtype))
    n_params = len(in_names)
    n_outs = len(out_avals)
    all_names = in_names + out_names
    if partition_name is not None:
        all_names = all_names + [partition_name]

    def _body(*args):
        operands = list(args)
        if partition_name is not None:
            operands.append(bass2jax.partition_id_tensor())
        outs = bass2jax._bass_exec_p.bind(
            *operands,
            out_avals=tuple(out_avals),
            in_names=tuple(all_names),
            out_names=tuple(out_names),
            lowering_input_output_aliases=(),
            sim_require_finite=True,
            sim_require_nnan=True,
            nc=nc,
        )
        return tuple(outs)

    devices = jax.devices()[:NCORES]
    mesh = Mesh(np.asarray(devices), ("core",))
    in_specs = (PartitionSpec("core"),) * (n_params + n_outs)
    out_specs = (PartitionSpec("core"),) * n_outs
    donate = tuple(range(n_params, n_params + n_outs))
    sharded = jax.jit(
        shard_map(_body, mesh=mesh, in_specs=in_specs, out_specs=out_specs,
                  check_rep=False),
        donate_argnums=donate,
        keep_unused=True,
    )
    sharding = NamedSharding(mesh, PartitionSpec("core"))
    runner = dict(
        fn=sharded, in_names=in_names, out_names=out_names,
        zero_outs=zero_outs, sharding=sharding, n_params=n_params,
    )
    _cached["runner"] = runner
    return runner


def _run_spmd(in_maps):
    import jax
    r = _make_runner()
    per_core = [[_np_asarray(m[nm]) for nm in r["in_names"]] for m in in_maps]
    concat_in = [
        np.concatenate([per_core[c][i] for c in range(NCORES)], axis=0)
        for i in range(len(r["in_names"]))
    ]
    concat_zeros = [
        np.zeros((NCORES * z.shape[0], *z.shape[1:]), z.dtype)
        for z in r["zero_outs"]
    ]
    outs = r["fn"](*concat_in, *concat_zeros)
    res = []
    for c in range(NCORES):
        d = {}
        for i, nm in enumerate(r["out_names"]):
            d[nm] = np.asarray(outs[i]).reshape(NCORES, -1)[c]
        res.append(d)
    return res


def _np_asarray(x):
    return np.asarray(x)

